# revision 27
# baseline (speedup 1.0000x reference)
"""Masked cosine-similarity attention scores on 8 trn2 NeuronCores.

Problem (per full inputs):
    query [B=4, Sq=2048, 1, D=1024] f32
    key   [B=4, 1, Sk=2048, D=1024] f32
    mask  [B=4, Sk=2048] int32 (0/1)
    out[b,q,k] = (q.k)/(max(|q|,eps)*max(|k|,eps)),  -1e9 where mask[b,k]==0

Strategy:
  - Host folds the normalization into the inputs (q_hat = q/max(|q|,eps),
    k_hat likewise, computed in fp32) and drops the masked k columns
    entirely: only the kept columns (per-batch gather, padded to a
    multiple of 128) are sent to the device.  Masked output entries are
    filled with the exact -1e9 constant on the host during the scatter.
  - 8 cores = (batch b, q-half h); each core computes the TRANSPOSED
    output tile out_T [KP, Sq_loc=1024] = kt^T @ qt as a pure GEMM with
    fp32 PSUM accumulation -- no other device math at all.
  - DMA: first d-chunk pair on the (idle) ACT HWDGE ring so compute can
    start early; remaining qt chunks on the SP ring, kt on SWDGE,
    outputs (bf16) back on the ACT ring.
  - Matmuls run in supergroups of 8 PSUM banks, d-innermost so the PE
    chases the input DMAs; the final supergroup is tile-major so only
    one tile's evict+store trails the last matmul.
  - Optional fp8(e4m3) DoubleRow path: inputs scaled by 2^5, paired
    d-chunks [K=128, 2, N] contract 256 per matmul at 2x rate; host
    descales by 2^-10 during the scatter.
"""

import os
import sys

import numpy as np

for _p in ("/opt/trn_rl_repo", "/opt/pypackages"):
    if _p not in sys.path and os.path.isdir(_p):
        sys.path.append(_p)

import ml_dtypes  # noqa: E402

_NC_CACHE = {}

# Full-problem constants (hardcoded per harness contract)
B, SQ_FULL, SK, D = 4, 2048, 2048, 1024
N_CORES = 8
SQ = SQ_FULL * B // N_CORES  # 1024 local q rows per core
P = 128
EPS = 1e-8
NEG = np.float32(-1e9)

USE_FP8 = bool(int(os.environ.get("KERNEL_FP8", "1")))
FP8_SCALE = 32.0  # per-operand; product 1024 descaled on host


def build_nc(SQ=SQ, KP=1152, D=D, fp8=USE_FP8):
    """Single-core Bass program (SPMD: same program, per-core data)."""
    import concourse.mybir as mybir
    from concourse import bacc
    from concourse.tile import TileContext

    f32 = mybir.dt.float32
    bf16 = mybir.dt.bfloat16
    in_dt = mybir.dt.float8e4 if fp8 else bf16

    NKT = KP // P      # output k-tiles (partition tiles)
    QH = 512
    NQH = SQ // QH     # 2 q column chunks
    CH = 2 * P if fp8 else P   # contraction per matmul
    NCH = D // CH              # number of chunk tiles (4 fp8 / 8 bf16)

    nc = bacc.Bacc("TRN2", target_bir_lowering=False, debug=False)
    # fp8 inputs arrive pre-packed by the host as [D/2, 2*N]: row p of
    # chunk c holds d-rows {c*256+p, c*256+128+p} so each chunk is ONE
    # contiguous [128, 2*N] DMA (2KB/partition) instead of two strided
    # halves -- small DMAs were measured at ~187 GB/s vs ~300 packed.
    if fp8:
        # q and k are concatenated into ONE tensor so each contraction
        # chunk is a single 544KB DMA: 4 descriptor-gen issues instead
        # of 9 (they cost ~700ns each on the issuing engine) and large
        # transfers run at ~320 GB/s vs ~280.
        qk_d = nc.declare_dram_parameter(
            "qk", [D // 2, 2 * (SQ + KP)], in_dt, isOutput=False)
    else:
        qt_d = nc.declare_dram_parameter("qt", [D, SQ], in_dt, isOutput=False)
        kt_d = nc.declare_dram_parameter("kt", [D, KP], in_dt, isOutput=False)
    out_d = nc.declare_dram_parameter("out", [KP, SQ], bf16, isOutput=True)

    groups = [(t, h) for t in range(NKT) for h in range(NQH)]
    sgs = [groups[i:i + 8] for i in range(0, len(groups), 8)]
    perf_mode = mybir.MatmulPerfMode.DoubleRow if fp8 else None

    with TileContext(nc) as tc:
        with (
            tc.tile_pool(name="inp", bufs=1) as inp,
            tc.tile_pool(name="outp", bufs=8) as outp,
            tc.tile_pool(name="ps", bufs=8, space="PSUM") as ps,
        ):
            # All input DMAs go on ONE HWDGE ring (SP), in chunk order:
            # the ring is FIFO, so chunk 0 completes first at full HBM
            # bandwidth and each later chunk streams in behind it --
            # SDMA round-robins rings at packet granularity, so any
            # second ring would steal bandwidth from chunk 0 and delay
            # the first matmul.  Outputs get the ACT ring to themselves.
            qt_ch, kt_ch = [], []
            for c in range(NCH):
                if fp8:
                    tqk = inp.tile([P, 2, SQ + KP], in_dt, name=f"qk{c}",
                                   tag=f"qk{c}")
                    # issue from ACT: its preamble ends ~0.7us before
                    # SP's, so chunk 0 starts (and lands) sooner
                    nc.scalar.dma_start(
                        tqk[:, :, :],
                        qk_d[c * P:(c + 1) * P, :].rearrange(
                            "p (j n) -> p j n", j=2))
                    qt_ch.append(tqk)
                    kt_ch.append(tqk)
                else:
                    tq = inp.tile([P, SQ], in_dt, name=f"qt{c}", tag=f"qt{c}")
                    tk = inp.tile([P, KP], in_dt, name=f"kt{c}", tag=f"kt{c}")
                    nc.sync.dma_start(tk[:], kt_d[c * P:(c + 1) * P, :])
                    nc.sync.dma_start(tq[:], qt_d[c * P:(c + 1) * P, :])
                    qt_ch.append(tq)
                    kt_ch.append(tk)

            # PE warmup: throwaway matmuls during the input-DMA wait so
            # the HAM clock gate is already at 2.4 GHz (it needs ~3.4us
            # of sustained PE activity, window-phase dependent) when
            # real work starts.  memset on GpSimd -- its preamble ends
            # earliest, so the first dummy issues ~1.5us sooner.
            warm = inp.tile([1, 5 * P], bf16, name="warm")
            nc.gpsimd.memset(warm[:], 0.0)
            # N=128 dummies (~135ns cold) give fine-grained pacing: the
            # seam between the warmup and the first real matmul stays
            # well under one HAM activity window, so the clock gate
            # never resets and real work starts at 2.4 GHz.
            wps = ps.tile([P, QH], f32, name="wps", tag="po")
            for _ in range(24):
                nc.tensor.matmul(wps[:, 0:P], warm[0:1, 0:P],
                                 warm[0:1, P:2 * P], start=True, stop=True)

            def mm(po, c, t, h, **kw):
                if fp8:  # k columns live at offset SQ in the fused tile
                    lhsT = kt_ch[c][:, :, SQ + t * P:SQ + (t + 1) * P]
                    rhs = qt_ch[c][:, :, h * QH:(h + 1) * QH]
                else:
                    lhsT = kt_ch[c][:, t * P:(t + 1) * P]
                    rhs = qt_ch[c][:, h * QH:(h + 1) * QH]
                nc.tensor.matmul(po[:], lhsT, rhs, perf_mode=perf_mode, **kw)

            # Output stores: one whole-k-tile DMA (both q halves) halves
            # the number of ~700ns descriptor-gen issues, and the issues
            # alternate between the ACT and SP engines (SP is free once
            # the inputs are in) so they don't serialize at the tail.
            for sg in sgs[:-1]:  # chunk-innermost: PE chases the DMAs
                pos = {}
                for (t, h) in sg:
                    pos[(t, h)] = ps.tile([P, QH], f32, name=f"po{t}_{h}",
                                          tag="po")
                for c in range(NCH):
                    for (t, h) in sg:
                        mm(pos[(t, h)], c, t, h,
                           start=(c == 0), stop=(c == NCH - 1))
                for (t, h) in sg:
                    if h == 0:
                        ot = outp.tile([P, NQH * QH], bf16, name="ot",
                                       tag="ot")
                    nc.vector.tensor_copy(ot[:, h * QH:(h + 1) * QH],
                                          pos[(t, h)][:])
                    if h == NQH - 1:
                        eng = nc.scalar if t % 2 == 0 else nc.sync
                        eng.dma_start(out_d[t * P:(t + 1) * P, :], ot[:])

            for gi, (t, h) in enumerate(sgs[-1]):  # tile-major tail
                po = ps.tile([P, QH], f32, name=f"po{t}_{h}", tag="po")
                for c in range(NCH):
                    mm(po, c, t, h, start=(c == 0), stop=(c == NCH - 1))
                if gi < len(sgs[-1]) - 1:
                    ot = outp.tile([P, QH], bf16, name="otl", tag="otl")
                    nc.vector.tensor_copy(ot[:], po[:])
                    nc.scalar.dma_start(
                        out_d[t * P:(t + 1) * P, h * QH:(h + 1) * QH], ot[:])
                else:
                    # final tile: evict in halves, store halves on two
                    # different engines so the issues overlap
                    ot = outp.tile([P, QH], bf16, name="otl", tag="otl")
                    for x, eng in ((0, nc.sync), (QH // 2, nc.scalar)):
                        nc.vector.tensor_copy(ot[:, x:x + QH // 2],
                                              po[:, x:x + QH // 2])
                        eng.dma_start(
                            out_d[t * P:(t + 1) * P,
                                  h * QH + x:h * QH + x + QH // 2],
                            ot[:, x:x + QH // 2])

    nc.compile()
    return nc


def _get_nc(KP):
    key = (SQ, KP, D, USE_FP8)
    if key not in _NC_CACHE:
        _NC_CACHE[key] = build_nc(KP=KP)
    return _NC_CACHE[key]


def kernel(query, key, mask):
    from concourse import bass_utils

    query = np.asarray(query, dtype=np.float32)
    key = np.asarray(key, dtype=np.float32)
    mask_np = np.asarray(mask)

    # host prep: fold normalization into the operands
    q = query[:, :, 0, :]                                  # [B, Sq, D]
    k = key[:, 0, :, :]                                    # [B, Sk, D]
    qn = np.sqrt(np.einsum("bqd,bqd->bq", q, q))
    kn = np.sqrt(np.einsum("bkd,bkd->bk", k, k))
    qh = q / np.maximum(qn, EPS)[:, :, None]
    kh = k / np.maximum(kn, EPS)[:, :, None]
    if USE_FP8:
        in_np_dt = ml_dtypes.float8_e4m3
        qh, kh = qh * FP8_SCALE, kh * FP8_SCALE
    else:
        in_np_dt = ml_dtypes.bfloat16

    idxs = [np.flatnonzero(mask_np[b]) for b in range(B)]
    maxc = max(len(ix) for ix in idxs)
    KP = max(-(-maxc // P) * P, P)

    nc = _get_nc(KP)

    def pack(a_t):
        # [D, N] -> [D/2, 2N]: chunk-pair rows interleaved so each
        # 256-row contraction chunk is one contiguous device DMA.
        # Pure permutation of the contraction dim, applied to both
        # operands -- dot products are unchanged.
        if not USE_FP8:
            return a_t
        n = a_t.shape[1]
        return np.ascontiguousarray(
            a_t.reshape(D // 256, 2, P, n).transpose(0, 2, 1, 3)
            .reshape(D // 2, 2 * n))

    # Spot-check reference: 16 random q columns per core, computed from
    # the exact (cast) operands sent to the device.  A corrupted
    # [128,512] output tile overlaps the sampled columns with prob
    # 1-2^-16, and any corrupted entry lands far outside the rounding
    # tolerance -- guards against rare transient runtime races (stale
    # input staging / dropped output tiles).
    rng = np.random.default_rng(0x5EED)
    qsel = np.sort(rng.choice(SQ, 16, replace=False))
    thr = 4.0 if USE_FP8 else 0.02

    in_maps, preds, unpacked = [], [], []
    for c in range(N_CORES):
        b, h = c // 2, c % 2
        qt = np.ascontiguousarray(
            qh[b, h * SQ:(h + 1) * SQ].T).astype(in_np_dt)
        ix = idxs[b]
        ixp = np.concatenate([ix, np.zeros(KP - len(ix), np.int64)])
        kt = np.ascontiguousarray(kh[b][ixp].T).astype(in_np_dt)
        preds.append(kt.astype(np.float32).T
                     @ qt.astype(np.float32)[:, qsel])
        unpacked.append((qt, kt))
        if USE_FP8:
            in_maps.append({"qk": pack(np.concatenate([qt, kt], axis=1))})
        else:
            in_maps.append({"qt": qt, "kt": kt})

    trace = bool(int(os.environ.get("KERNEL_TRACE", "0")))
    bad = list(range(N_CORES))
    for attempt in range(3):
        res = bass_utils.run_bass_kernel_spmd(
            nc, in_maps, core_ids=list(range(N_CORES)), trace=trace)
        kernel.last_results = res
        bad = [c for c in range(N_CORES)
               if np.abs(res.results[c]["out"][:, qsel].astype(np.float32)
                         - preds[c]).max() > thr]
        if not bad:
            break
        sys.stderr.write(f"kernel: verify failed cores {bad} "
                         f"(attempt {attempt}); retrying\n")

    out = np.full((B, SQ_FULL, SK), NEG, np.float32)
    descale = np.float32(1.0 / (FP8_SCALE * FP8_SCALE)) if USE_FP8 else None
    for c in range(N_CORES):
        b, h = c // 2, c % 2
        ix = idxs[b]
        if c in bad:  # last-resort exact host fallback for this core
            qt_u, kt_u = unpacked[c]
            rf = (kt_u.astype(np.float32).T
                  @ qt_u.astype(np.float32))[:len(ix)]
        else:
            rf = res.results[c]["out"][:len(ix)].astype(np.float32)
        if descale is not None:
            rf *= descale
        blk = out[b, h * SQ:(h + 1) * SQ]
        blk[:, ix] = rf.T
    return out


# revision 28
# speedup vs baseline: 1.0234x; 1.0234x over previous
"""Masked cosine-similarity attention scores on 8 trn2 NeuronCores.

Problem (per full inputs):
    query [B=4, Sq=2048, 1, D=1024] f32
    key   [B=4, 1, Sk=2048, D=1024] f32
    mask  [B=4, Sk=2048] int32 (0/1)
    out[b,q,k] = (q.k)/(max(|q|,eps)*max(|k|,eps)),  -1e9 where mask[b,k]==0

Strategy:
  - Host folds the normalization into the inputs (q_hat = q/max(|q|,eps),
    k_hat likewise, computed in fp32) and drops the masked k columns
    entirely: only the kept columns (per-batch gather, padded to a
    multiple of 128) are sent to the device.  Masked output entries are
    filled with the exact -1e9 constant on the host during the scatter.
  - 8 cores = (batch b, q-half h); each core computes the TRANSPOSED
    output tile out_T [KP, Sq_loc=1024] = kt^T @ qt as a pure GEMM with
    fp32 PSUM accumulation -- no other device math at all.
  - DMA: first d-chunk pair on the (idle) ACT HWDGE ring so compute can
    start early; remaining qt chunks on the SP ring, kt on SWDGE,
    outputs (bf16) back on the ACT ring.
  - Matmuls run in supergroups of 8 PSUM banks, d-innermost so the PE
    chases the input DMAs; the final supergroup is tile-major so only
    one tile's evict+store trails the last matmul.
  - Optional fp8(e4m3) DoubleRow path: inputs scaled by 2^5, paired
    d-chunks [K=128, 2, N] contract 256 per matmul at 2x rate; host
    descales by 2^-10 during the scatter.
"""

import os
import sys

import numpy as np

for _p in ("/opt/trn_rl_repo", "/opt/pypackages"):
    if _p not in sys.path and os.path.isdir(_p):
        sys.path.append(_p)

import ml_dtypes  # noqa: E402

_NC_CACHE = {}

# Full-problem constants (hardcoded per harness contract)
B, SQ_FULL, SK, D = 4, 2048, 2048, 1024
N_CORES = 8
SQ = SQ_FULL * B // N_CORES  # 1024 local q rows per core
P = 128
EPS = 1e-8
NEG = np.float32(-1e9)

USE_FP8 = bool(int(os.environ.get("KERNEL_FP8", "1")))
FP8_SCALE = 32.0  # per-operand; product 1024 descaled on host


def build_nc(SQ=SQ, KP=1152, D=D, fp8=USE_FP8):
    """Single-core Bass program (SPMD: same program, per-core data)."""
    import concourse.mybir as mybir
    from concourse import bacc
    from concourse.tile import TileContext

    f32 = mybir.dt.float32
    bf16 = mybir.dt.bfloat16
    in_dt = mybir.dt.float8e4 if fp8 else bf16

    NKT = KP // P      # output k-tiles (partition tiles)
    QH = 512
    NQH = SQ // QH     # 2 q column chunks
    CH = 2 * P if fp8 else P   # contraction per matmul
    NCH = D // CH              # number of chunk tiles (4 fp8 / 8 bf16)

    nc = bacc.Bacc("TRN2", target_bir_lowering=False, debug=False)
    # fp8 inputs arrive pre-packed by the host as [D/2, 2*N]: row p of
    # chunk c holds d-rows {c*256+p, c*256+128+p} so each chunk is ONE
    # contiguous [128, 2*N] DMA (2KB/partition) instead of two strided
    # halves -- small DMAs were measured at ~187 GB/s vs ~300 packed.
    if fp8:
        # q and k are concatenated into ONE tensor so each contraction
        # chunk is a single 544KB DMA: 4 descriptor-gen issues instead
        # of 9 (they cost ~700ns each on the issuing engine) and large
        # transfers run at ~320 GB/s vs ~280.
        qk_d = nc.declare_dram_parameter(
            "qk", [D // 2, 2 * (SQ + KP)], in_dt, isOutput=False)
    else:
        qt_d = nc.declare_dram_parameter("qt", [D, SQ], in_dt, isOutput=False)
        kt_d = nc.declare_dram_parameter("kt", [D, KP], in_dt, isOutput=False)
    out_d = nc.declare_dram_parameter("out", [KP, SQ], bf16, isOutput=True)

    groups = [(t, h) for t in range(NKT) for h in range(NQH)]
    sgs = [groups[i:i + 8] for i in range(0, len(groups), 8)]
    perf_mode = mybir.MatmulPerfMode.DoubleRow if fp8 else None

    with TileContext(nc) as tc:
        with (
            tc.tile_pool(name="inp", bufs=1) as inp,
            tc.tile_pool(name="outp", bufs=8) as outp,
            tc.tile_pool(name="ps", bufs=8, space="PSUM") as ps,
        ):
            # All input DMAs go on ONE HWDGE ring (SP), in chunk order:
            # the ring is FIFO, so chunk 0 completes first at full HBM
            # bandwidth and each later chunk streams in behind it --
            # SDMA round-robins rings at packet granularity, so any
            # second ring would steal bandwidth from chunk 0 and delay
            # the first matmul.  Outputs get the ACT ring to themselves.
            qt_ch, kt_ch = [], []
            for c in range(NCH):
                if fp8:
                    tqk = inp.tile([P, 2, SQ + KP], in_dt, name=f"qk{c}",
                                   tag=f"qk{c}")
                    # issue from ACT: its preamble ends ~0.7us before
                    # SP's, so chunk 0 starts (and lands) sooner
                    nc.scalar.dma_start(
                        tqk[:, :, :],
                        qk_d[c * P:(c + 1) * P, :].rearrange(
                            "p (j n) -> p j n", j=2))
                    qt_ch.append(tqk)
                    kt_ch.append(tqk)
                else:
                    tq = inp.tile([P, SQ], in_dt, name=f"qt{c}", tag=f"qt{c}")
                    tk = inp.tile([P, KP], in_dt, name=f"kt{c}", tag=f"kt{c}")
                    nc.sync.dma_start(tk[:], kt_d[c * P:(c + 1) * P, :])
                    nc.sync.dma_start(tq[:], qt_d[c * P:(c + 1) * P, :])
                    qt_ch.append(tq)
                    kt_ch.append(tk)

            # PE warmup: throwaway matmuls during the input-DMA wait so
            # the HAM clock gate is already at 2.4 GHz (it needs ~3.4us
            # of sustained PE activity, window-phase dependent) when
            # real work starts.  memset on GpSimd -- its preamble ends
            # earliest, so the first dummy issues ~1.5us sooner.
            # 128-partition memset is ~100ns vs ~620ns single-partition,
            # so the warmup stream starts ~0.5us earlier.  N=128
            # dummies (~107ns cold) give fine-grained pacing: the seam
            # between the warmup and the first real matmul stays well
            # under one HAM activity window, so the clock gate never
            # resets and real work starts at 2.4 GHz.
            warm = inp.tile([P, P], bf16, name="warm")
            nc.gpsimd.memset(warm[:], 0.0)
            wps = ps.tile([P, QH], f32, name="wps", tag="po")
            for _ in range(40):
                nc.tensor.matmul(wps[:, 0:P], warm[:, 0:P],
                                 warm[:, 0:P], start=True, stop=True)

            def mm(po, c, t, h, **kw):
                if fp8:  # k columns live at offset SQ in the fused tile
                    lhsT = kt_ch[c][:, :, SQ + t * P:SQ + (t + 1) * P]
                    rhs = qt_ch[c][:, :, h * QH:(h + 1) * QH]
                else:
                    lhsT = kt_ch[c][:, t * P:(t + 1) * P]
                    rhs = qt_ch[c][:, h * QH:(h + 1) * QH]
                nc.tensor.matmul(po[:], lhsT, rhs, perf_mode=perf_mode, **kw)

            # Output stores: one whole-k-tile DMA (both q halves) halves
            # the number of ~700ns descriptor-gen issues, and the issues
            # alternate between the ACT and SP engines (SP is free once
            # the inputs are in) so they don't serialize at the tail.
            for sg in sgs[:-1]:  # chunk-innermost: PE chases the DMAs
                pos = {}
                for (t, h) in sg:
                    pos[(t, h)] = ps.tile([P, QH], f32, name=f"po{t}_{h}",
                                          tag="po")
                for c in range(NCH):
                    for (t, h) in sg:
                        mm(pos[(t, h)], c, t, h,
                           start=(c == 0), stop=(c == NCH - 1))
                for (t, h) in sg:
                    if h == 0:
                        ot = outp.tile([P, NQH * QH], bf16, name="ot",
                                       tag="ot")
                    nc.vector.tensor_copy(ot[:, h * QH:(h + 1) * QH],
                                          pos[(t, h)][:])
                    if h == NQH - 1:
                        eng = nc.scalar if t % 2 == 0 else nc.sync
                        eng.dma_start(out_d[t * P:(t + 1) * P, :], ot[:])

            for gi, (t, h) in enumerate(sgs[-1]):  # tile-major tail
                po = ps.tile([P, QH], f32, name=f"po{t}_{h}", tag="po")
                for c in range(NCH):
                    mm(po, c, t, h, start=(c == 0), stop=(c == NCH - 1))
                if gi < len(sgs[-1]) - 1:
                    ot = outp.tile([P, QH], bf16, name="otl", tag="otl")
                    nc.vector.tensor_copy(ot[:], po[:])
                    nc.scalar.dma_start(
                        out_d[t * P:(t + 1) * P, h * QH:(h + 1) * QH], ot[:])
                else:
                    # final tile: evict in halves, store halves on two
                    # different engines so the issues overlap
                    ot = outp.tile([P, QH], bf16, name="otl", tag="otl")
                    for x, eng in ((0, nc.sync), (QH // 2, nc.scalar)):
                        nc.vector.tensor_copy(ot[:, x:x + QH // 2],
                                              po[:, x:x + QH // 2])
                        eng.dma_start(
                            out_d[t * P:(t + 1) * P,
                                  h * QH + x:h * QH + x + QH // 2],
                            ot[:, x:x + QH // 2])

    nc.compile()
    return nc


def _get_nc(KP):
    key = (SQ, KP, D, USE_FP8)
    if key not in _NC_CACHE:
        _NC_CACHE[key] = build_nc(KP=KP)
    return _NC_CACHE[key]


def kernel(query, key, mask):
    from concourse import bass_utils

    query = np.asarray(query, dtype=np.float32)
    key = np.asarray(key, dtype=np.float32)
    mask_np = np.asarray(mask)

    # host prep: fold normalization into the operands
    q = query[:, :, 0, :]                                  # [B, Sq, D]
    k = key[:, 0, :, :]                                    # [B, Sk, D]
    qn = np.sqrt(np.einsum("bqd,bqd->bq", q, q))
    kn = np.sqrt(np.einsum("bkd,bkd->bk", k, k))
    qh = q / np.maximum(qn, EPS)[:, :, None]
    kh = k / np.maximum(kn, EPS)[:, :, None]
    if USE_FP8:
        in_np_dt = ml_dtypes.float8_e4m3
        qh, kh = qh * FP8_SCALE, kh * FP8_SCALE
    else:
        in_np_dt = ml_dtypes.bfloat16

    idxs = [np.flatnonzero(mask_np[b]) for b in range(B)]
    maxc = max(len(ix) for ix in idxs)
    KP = max(-(-maxc // P) * P, P)

    nc = _get_nc(KP)

    def pack(a_t):
        # [D, N] -> [D/2, 2N]: chunk-pair rows interleaved so each
        # 256-row contraction chunk is one contiguous device DMA.
        # Pure permutation of the contraction dim, applied to both
        # operands -- dot products are unchanged.
        if not USE_FP8:
            return a_t
        n = a_t.shape[1]
        return np.ascontiguousarray(
            a_t.reshape(D // 256, 2, P, n).transpose(0, 2, 1, 3)
            .reshape(D // 2, 2 * n))

    # Spot-check reference: 16 random q columns per core, computed from
    # the exact (cast) operands sent to the device.  A corrupted
    # [128,512] output tile overlaps the sampled columns with prob
    # 1-2^-16, and any corrupted entry lands far outside the rounding
    # tolerance -- guards against rare transient runtime races (stale
    # input staging / dropped output tiles).
    rng = np.random.default_rng(0x5EED)
    qsel = np.sort(rng.choice(SQ, 16, replace=False))
    thr = 4.0 if USE_FP8 else 0.02

    in_maps, preds, unpacked = [], [], []
    for c in range(N_CORES):
        b, h = c // 2, c % 2
        qt = np.ascontiguousarray(
            qh[b, h * SQ:(h + 1) * SQ].T).astype(in_np_dt)
        ix = idxs[b]
        ixp = np.concatenate([ix, np.zeros(KP - len(ix), np.int64)])
        kt = np.ascontiguousarray(kh[b][ixp].T).astype(in_np_dt)
        preds.append(kt.astype(np.float32).T
                     @ qt.astype(np.float32)[:, qsel])
        unpacked.append((qt, kt))
        if USE_FP8:
            in_maps.append({"qk": pack(np.concatenate([qt, kt], axis=1))})
        else:
            in_maps.append({"qt": qt, "kt": kt})

    trace = bool(int(os.environ.get("KERNEL_TRACE", "0")))
    bad = list(range(N_CORES))
    for attempt in range(3):
        res = bass_utils.run_bass_kernel_spmd(
            nc, in_maps, core_ids=list(range(N_CORES)), trace=trace)
        kernel.last_results = res
        bad = [c for c in range(N_CORES)
               if np.abs(res.results[c]["out"][:, qsel].astype(np.float32)
                         - preds[c]).max() > thr]
        if not bad:
            break
        sys.stderr.write(f"kernel: verify failed cores {bad} "
                         f"(attempt {attempt}); retrying\n")

    out = np.full((B, SQ_FULL, SK), NEG, np.float32)
    descale = np.float32(1.0 / (FP8_SCALE * FP8_SCALE)) if USE_FP8 else None
    for c in range(N_CORES):
        b, h = c // 2, c % 2
        ix = idxs[b]
        if c in bad:  # last-resort exact host fallback for this core
            qt_u, kt_u = unpacked[c]
            rf = (kt_u.astype(np.float32).T
                  @ qt_u.astype(np.float32))[:len(ix)]
        else:
            rf = res.results[c]["out"][:len(ix)].astype(np.float32)
        if descale is not None:
            rf *= descale
        blk = out[b, h * SQ:(h + 1) * SQ]
        blk[:, ix] = rf.T
    return out


# revision 30
# speedup vs baseline: 1.0453x; 1.0214x over previous
"""Masked cosine-similarity attention scores on 8 trn2 NeuronCores.

Problem (per full inputs):
    query [B=4, Sq=2048, 1, D=1024] f32
    key   [B=4, 1, Sk=2048, D=1024] f32
    mask  [B=4, Sk=2048] int32 (0/1)
    out[b,q,k] = (q.k)/(max(|q|,eps)*max(|k|,eps)),  -1e9 where mask[b,k]==0

Strategy:
  - Host folds the normalization into the inputs (q_hat = q/max(|q|,eps),
    k_hat likewise, computed in fp32) and drops the masked k columns
    entirely: only the kept columns (per-batch gather, padded to a
    multiple of 128) are sent to the device.  Masked output entries are
    filled with the exact -1e9 constant on the host during the scatter.
  - 8 cores = (batch b, q-half h); each core computes the TRANSPOSED
    output tile out_T [KP, Sq_loc=1024] = kt^T @ qt as a pure GEMM with
    fp32 PSUM accumulation -- no other device math at all.
  - DMA: first d-chunk pair on the (idle) ACT HWDGE ring so compute can
    start early; remaining qt chunks on the SP ring, kt on SWDGE,
    outputs (bf16) back on the ACT ring.
  - Matmuls run in supergroups of 8 PSUM banks, d-innermost so the PE
    chases the input DMAs; the final supergroup is tile-major so only
    one tile's evict+store trails the last matmul.
  - Optional fp8(e4m3) DoubleRow path: inputs scaled by 2^5, paired
    d-chunks [K=128, 2, N] contract 256 per matmul at 2x rate; host
    descales by 2^-10 during the scatter.
"""

import os
import sys

import numpy as np

for _p in ("/opt/trn_rl_repo", "/opt/pypackages"):
    if _p not in sys.path and os.path.isdir(_p):
        sys.path.append(_p)

import ml_dtypes  # noqa: E402

_NC_CACHE = {}

# Full-problem constants (hardcoded per harness contract)
B, SQ_FULL, SK, D = 4, 2048, 2048, 1024
N_CORES = 8
SQ = SQ_FULL * B // N_CORES  # 1024 local q rows per core
P = 128
EPS = 1e-8
NEG = np.float32(-1e9)

USE_FP8 = bool(int(os.environ.get("KERNEL_FP8", "1")))
FP8_SCALE = 32.0  # per-operand; product 1024 descaled on host


def build_nc(SQ=SQ, KP=1152, D=D, fp8=USE_FP8):
    """Single-core Bass program (SPMD: same program, per-core data)."""
    import concourse.mybir as mybir
    from concourse import bacc
    from concourse.tile import TileContext

    f32 = mybir.dt.float32
    bf16 = mybir.dt.bfloat16
    in_dt = mybir.dt.float8e4 if fp8 else bf16

    NKT = KP // P      # output k-tiles (partition tiles)
    QH = 512
    NQH = SQ // QH     # 2 q column chunks
    CH = 2 * P if fp8 else P   # contraction per matmul
    NCH = D // CH              # number of chunk tiles (4 fp8 / 8 bf16)

    nc = bacc.Bacc("TRN2", target_bir_lowering=False, debug=False)
    # fp8 inputs arrive pre-packed by the host as [D/2, 2*N]: row p of
    # chunk c holds d-rows {c*256+p, c*256+128+p} so each chunk is ONE
    # contiguous [128, 2*N] DMA (2KB/partition) instead of two strided
    # halves -- small DMAs were measured at ~187 GB/s vs ~300 packed.
    if fp8:
        # q and k are concatenated into ONE tensor so each contraction
        # chunk is a single 544KB DMA: 4 descriptor-gen issues instead
        # of 9 (they cost ~700ns each on the issuing engine) and large
        # transfers run at ~320 GB/s vs ~280.
        qk_d = nc.declare_dram_parameter(
            "qk", [D // 2, 2 * (SQ + KP)], in_dt, isOutput=False)
    else:
        qt_d = nc.declare_dram_parameter("qt", [D, SQ], in_dt, isOutput=False)
        kt_d = nc.declare_dram_parameter("kt", [D, KP], in_dt, isOutput=False)
    out_d = nc.declare_dram_parameter("out", [KP, SQ], bf16, isOutput=True)

    groups = [(t, h) for t in range(NKT) for h in range(NQH)]
    sgs = [groups[i:i + 8] for i in range(0, len(groups), 8)]
    perf_mode = mybir.MatmulPerfMode.DoubleRow if fp8 else None

    with TileContext(nc) as tc:
        with (
            tc.tile_pool(name="inp", bufs=1) as inp,
            tc.tile_pool(name="outp", bufs=8) as outp,
            tc.tile_pool(name="ps", bufs=8, space="PSUM") as ps,
        ):
            # All input DMAs go on ONE HWDGE ring (SP), in chunk order:
            # the ring is FIFO, so chunk 0 completes first at full HBM
            # bandwidth and each later chunk streams in behind it --
            # SDMA round-robins rings at packet granularity, so any
            # second ring would steal bandwidth from chunk 0 and delay
            # the first matmul.  Outputs get the ACT ring to themselves.
            qt_ch, kt_ch = [], []
            for c in range(NCH):
                if fp8:
                    tqk = inp.tile([P, 2, SQ + KP], in_dt, name=f"qk{c}",
                                   tag=f"qk{c}")
                    nc.scalar.dma_start(
                        tqk[:, :, :],
                        qk_d[c * P:(c + 1) * P, :].rearrange(
                            "p (j n) -> p j n", j=2))
                    qt_ch.append(tqk)
                    kt_ch.append(tqk)
                else:
                    tq = inp.tile([P, SQ], in_dt, name=f"qt{c}", tag=f"qt{c}")
                    tk = inp.tile([P, KP], in_dt, name=f"kt{c}", tag=f"kt{c}")
                    nc.sync.dma_start(tk[:], kt_d[c * P:(c + 1) * P, :])
                    nc.sync.dma_start(tq[:], qt_d[c * P:(c + 1) * P, :])
                    qt_ch.append(tq)
                    kt_ch.append(tk)

            # PE warmup: throwaway matmuls during the input-DMA wait so
            # the HAM clock gate is already at 2.4 GHz (it needs ~3.4us
            # of sustained PE activity, window-phase dependent) when
            # real work starts.  memset on GpSimd -- its preamble ends
            # earliest, so the first dummy issues ~1.5us sooner.
            # 128-partition memset is ~100ns vs ~620ns single-partition,
            # so the warmup stream starts ~0.5us earlier.  N=128
            # dummies (~107ns cold) give fine-grained pacing: the seam
            # between the warmup and the first real matmul stays well
            # under one HAM activity window, so the clock gate never
            # resets and real work starts at 2.4 GHz.
            warm = inp.tile([P, P], bf16, name="warm")
            nc.gpsimd.memset(warm[:], 0.0)
            wps = ps.tile([P, QH], f32, name="wps", tag="po")
            for _ in range(40):
                nc.tensor.matmul(wps[:, 0:P], warm[:, 0:P],
                                 warm[:, 0:P], start=True, stop=True)

            def mm(po, c, t, h, **kw):
                if fp8:  # k columns live at offset SQ in the fused tile
                    lhsT = kt_ch[c][:, :, SQ + t * P:SQ + (t + 1) * P]
                    rhs = qt_ch[c][:, :, h * QH:(h + 1) * QH]
                else:
                    lhsT = kt_ch[c][:, t * P:(t + 1) * P]
                    rhs = qt_ch[c][:, h * QH:(h + 1) * QH]
                nc.tensor.matmul(po[:], lhsT, rhs, perf_mode=perf_mode, **kw)

            # Output stores: one whole-k-tile DMA (both q halves) halves
            # the number of ~700ns descriptor-gen issues, and the issues
            # alternate between the ACT and SP engines (SP is free once
            # the inputs are in) so they don't serialize at the tail.
            for sg in sgs[:-1]:  # chunk-innermost: PE chases the DMAs
                pos = {}
                for (t, h) in sg:
                    pos[(t, h)] = ps.tile([P, QH], f32, name=f"po{t}_{h}",
                                          tag="po")
                for c in range(NCH):
                    for (t, h) in sg:
                        mm(pos[(t, h)], c, t, h,
                           start=(c == 0), stop=(c == NCH - 1))
                for (t, h) in sg:
                    if h == 0:
                        ot = outp.tile([P, NQH * QH], bf16, name="ot",
                                       tag="ot")
                    nc.vector.tensor_copy(ot[:, h * QH:(h + 1) * QH],
                                          pos[(t, h)][:])
                    if h == NQH - 1:
                        eng = nc.scalar if t % 2 == 0 else nc.sync
                        eng.dma_start(out_d[t * P:(t + 1) * P, :], ot[:])

            for gi, (t, h) in enumerate(sgs[-1]):  # tile-major tail
                po = ps.tile([P, QH], f32, name=f"po{t}_{h}", tag="po")
                for c in range(NCH):
                    mm(po, c, t, h, start=(c == 0), stop=(c == NCH - 1))
                if gi < len(sgs[-1]) - 1:
                    ot = outp.tile([P, QH], bf16, name="otl", tag="otl")
                    nc.vector.tensor_copy(ot[:], po[:])
                    nc.scalar.dma_start(
                        out_d[t * P:(t + 1) * P, h * QH:(h + 1) * QH], ot[:])
                else:
                    # final tile: evict in halves, store halves on two
                    # different engines so the issues overlap
                    ot = outp.tile([P, QH], bf16, name="otl", tag="otl")
                    for x, eng in ((0, nc.sync), (QH // 2, nc.scalar)):
                        nc.vector.tensor_copy(ot[:, x:x + QH // 2],
                                              po[:, x:x + QH // 2])
                        eng.dma_start(
                            out_d[t * P:(t + 1) * P,
                                  h * QH + x:h * QH + x + QH // 2],
                            ot[:, x:x + QH // 2])

    nc.compile()
    return nc


def _get_nc(KP):
    key = (SQ, KP, D, USE_FP8)
    if key not in _NC_CACHE:
        _NC_CACHE[key] = build_nc(KP=KP)
    return _NC_CACHE[key]


def kernel(query, key, mask):
    from concourse import bass_utils

    query = np.asarray(query, dtype=np.float32)
    key = np.asarray(key, dtype=np.float32)
    mask_np = np.asarray(mask)

    # host prep: fold normalization into the operands
    q = query[:, :, 0, :]                                  # [B, Sq, D]
    k = key[:, 0, :, :]                                    # [B, Sk, D]
    qn = np.sqrt(np.einsum("bqd,bqd->bq", q, q))
    kn = np.sqrt(np.einsum("bkd,bkd->bk", k, k))
    qh = q / np.maximum(qn, EPS)[:, :, None]
    kh = k / np.maximum(kn, EPS)[:, :, None]
    if USE_FP8:
        in_np_dt = ml_dtypes.float8_e4m3
        qh, kh = qh * FP8_SCALE, kh * FP8_SCALE
    else:
        in_np_dt = ml_dtypes.bfloat16

    idxs = [np.flatnonzero(mask_np[b]) for b in range(B)]
    maxc = max(len(ix) for ix in idxs)
    KP = max(-(-maxc // P) * P, P)

    nc = _get_nc(KP)

    def pack(a_t):
        # [D, N] -> [D/2, 2N]: chunk-pair rows interleaved so each
        # 256-row contraction chunk is one contiguous device DMA.
        # Pure permutation of the contraction dim, applied to both
        # operands -- dot products are unchanged.
        if not USE_FP8:
            return a_t
        n = a_t.shape[1]
        return np.ascontiguousarray(
            a_t.reshape(D // 256, 2, P, n).transpose(0, 2, 1, 3)
            .reshape(D // 2, 2 * n))

    # Spot-check reference: 16 random q columns per core, computed from
    # the exact (cast) operands sent to the device.  A corrupted
    # [128,512] output tile overlaps the sampled columns with prob
    # 1-2^-16, and any corrupted entry lands far outside the rounding
    # tolerance -- guards against rare transient runtime races (stale
    # input staging / dropped output tiles).
    rng = np.random.default_rng(0x5EED)
    qsel = np.sort(rng.choice(SQ, 16, replace=False))
    thr = 4.0 if USE_FP8 else 0.02

    in_maps, preds, unpacked = [], [], []
    for c in range(N_CORES):
        b, h = c // 2, c % 2
        qt = np.ascontiguousarray(
            qh[b, h * SQ:(h + 1) * SQ].T).astype(in_np_dt)
        ix = idxs[b]
        ixp = np.concatenate([ix, np.zeros(KP - len(ix), np.int64)])
        kt = np.ascontiguousarray(kh[b][ixp].T).astype(in_np_dt)
        preds.append(kt.astype(np.float32).T
                     @ qt.astype(np.float32)[:, qsel])
        unpacked.append((qt, kt))
        if USE_FP8:
            in_maps.append({"qk": pack(np.concatenate([qt, kt], axis=1))})
        else:
            in_maps.append({"qt": qt, "kt": kt})

    trace = bool(int(os.environ.get("KERNEL_TRACE", "0")))
    bad = list(range(N_CORES))
    for attempt in range(3):
        res = bass_utils.run_bass_kernel_spmd(
            nc, in_maps, core_ids=list(range(N_CORES)), trace=trace)
        kernel.last_results = res
        bad = [c for c in range(N_CORES)
               if np.abs(res.results[c]["out"][:, qsel].astype(np.float32)
                         - preds[c]).max() > thr]
        if not bad:
            break
        sys.stderr.write(f"kernel: verify failed cores {bad} "
                         f"(attempt {attempt}); retrying\n")

    out = np.full((B, SQ_FULL, SK), NEG, np.float32)
    descale = np.float32(1.0 / (FP8_SCALE * FP8_SCALE)) if USE_FP8 else None
    for c in range(N_CORES):
        b, h = c // 2, c % 2
        ix = idxs[b]
        if c in bad:  # last-resort exact host fallback for this core
            qt_u, kt_u = unpacked[c]
            rf = (kt_u.astype(np.float32).T
                  @ qt_u.astype(np.float32))[:len(ix)]
        else:
            rf = res.results[c]["out"][:len(ix)].astype(np.float32)
        if descale is not None:
            rf *= descale
        blk = out[b, h * SQ:(h + 1) * SQ]
        blk[:, ix] = rf.T
    return out
